# revision 30
# baseline (speedup 1.0000x reference)
"""TRN2 Bass/Tile kernel for nn_Block_19756849561899 (pre-LN transformer
block: LN -> MHA -> residual -> LN -> MLP(gelu) -> residual).

Self-contained: kernel(**inputs) takes the full fp32 tensors, shards work
across 8 NeuronCores (one batch per core-pair; each core owns half the
sequence as queries and redundantly builds K/V for its batch), compiles a
Bass/Tile program once per process, runs it SPMD, and reassembles the full
output.

Structure (v2): the attention phase is ACT-bound (back-to-back EXPs pace
it), so MLP/proj PE work for query-tile j-1 is emitted to overlap the
attention of j. Scores for the two heads of a pair run concurrently in
disjoint PE row groups (contract dim is 64). All transposes ride the idle
DMA engines (xbar transpose) instead of PE+PSUM.
"""

import contextlib

import numpy as np
import ml_dtypes

import concourse.bass as bass
import concourse.mybir as mybir
import concourse.tile as tile
from concourse.masks import make_identity

fp32 = mybir.dt.float32
bf16 = mybir.dt.bfloat16
fp8 = mybir.dt.float8e4
AF = mybir.ActivationFunctionType
ALU = mybir.AluOpType

C = 384
CS = 3          # C / 128
H = 6
HP = 3          # head pairs
DH = 64
HID = 1536
KS = 12         # HID / 128
VW = 72         # padded V row width (DoubleRow needs 16B-aligned pair stride)
WSCALE = 16.0   # fp8 fc2 weight scaling (avoids e4m3 denormals)
EPS = 1e-6
NBIAS = 24
BUST = "b8"     # bump on every IR change: the NEFF cache key can miss
                # SBUF-internal IR edits, so keep the io signature fresh


def build(nc, SEQ=2048, act_fn=AF.Gelu):
    TT = SEQ // 128          # token tiles over full sequence
    QTT = TT // 2            # token tiles in own (query) half
    QLEN = SEQ // 2
    QF = min(512, QLEN)      # q free-dim tile
    NJ = QLEN // QF
    NF = min(512, SEQ)       # seq free-dim tile for K^T build
    NN = SEQ // NF
    NB = QF // 128           # token blocks per q-tile
    NPV = TT // 2            # PV DoubleRow kt-pairs

    xin = nc.dram_tensor("xin", [SEQ, C], fp32, kind="ExternalInput")
    wqk_d = nc.dram_tensor("wqk", [128, CS, 768], bf16, kind="ExternalInput")
    wv_d = nc.dram_tensor("wv", [128, CS, C], bf16, kind="ExternalInput")
    wp_d = nc.dram_tensor("wp", [128, CS, C], bf16, kind="ExternalInput")
    wf1_d = nc.dram_tensor("wf1", [128, CS, HID], bf16, kind="ExternalInput")
    wf2_d = nc.dram_tensor("wf2", [128, KS, C], fp8, kind="ExternalInput")
    bias_d = nc.dram_tensor("bias", [128, NBIAS], fp32, kind="ExternalInput")
    bv_d = nc.dram_tensor("bv", [1, C], fp32, kind="ExternalInput")
    yout = nc.dram_tensor("yout", [QLEN, C], fp32, kind="ExternalOutput")
    bust_d = nc.dram_tensor(f"bustin_{BUST}", [1, 8], fp32, kind="ExternalInput")
    bust_o = nc.dram_tensor(f"bustout_{BUST}", [1, 8], fp32, kind="ExternalOutput")

    xin_t = xin.ap().rearrange("(t p) c -> p t c", p=128)     # [128, TT, C]
    yout_t = yout.ap().rearrange("(t p) c -> p t c", p=128)   # [128, QTT, C]

    with tile.TileContext(nc) as tc, contextlib.ExitStack() as ctx:
        per = ctx.enter_context(tc.tile_pool(name="per", bufs=1))
        ldx = ctx.enter_context(tc.tile_pool(name="ldx", bufs=6))
        xnp = ctx.enter_context(tc.tile_pool(name="xnp", bufs=6))
        echp = ctx.enter_context(tc.tile_pool(name="echp", bufs=2))
        rzp = ctx.enter_context(tc.tile_pool(name="rzp", bufs=3))
        ytp = ctx.enter_context(tc.tile_pool(name="ytp", bufs=4))
        hfp = ctx.enter_context(tc.tile_pool(name="hfp", bufs=2))
        sta = ctx.enter_context(tc.tile_pool(name="sta", bufs=1))
        # PSUM banks: pss 2x2 (S head-pair chunks) + pom 2x1 (PV accum)
        #           + psg 2x1 (warm/QK/V/proj/MLP/transposes) = 8
        pss = ctx.enter_context(tc.tile_pool(name="pss", bufs=2, space="PSUM"))
        pom = ctx.enter_context(tc.tile_pool(name="pom", bufs=2, space="PSUM"))
        psg = ctx.enter_context(tc.tile_pool(name="psg", bufs=2, space="PSUM"))

        bust_t = per.tile([1, 8], fp32)
        nc.sync.dma_start(bust_t[:], bust_d.ap())
        nc.sync.dma_start(bust_o.ap(), bust_t[:])

        wqk = per.tile([128, CS, 768], bf16)
        wv = per.tile([128, CS, C], bf16)
        bias = per.tile([128, NBIAS], fp32)
        bv = per.tile([128, C], fp32)
        wp = per.tile([128, CS, C], bf16)
        wf1 = per.tile([128, CS, HID], bf16)
        wf2 = per.tile([128, KS, C], fp8)
        ident = per.tile([128, 128], bf16)
        make_identity(nc, ident)

        x_own = per.tile([128, QTT, C], fp32)
        x_oth = per.tile([128, QTT, C], fp32)
        x2 = per.tile([128, QTT, C], fp32)
        KT = per.tile([128, HP, SEQ], bf16)
        QT = per.tile([128, HP, QLEN], bf16)
        Vsb = per.tile([128, TT, H, VW], fp8)
        xnT = per.tile([128, CS, SEQ], bf16)
        xn2T = per.tile([128, CS, QLEN], bf16)
        AT = per.tile([128, HP, QLEN], bf16)

        nc.vector.memset(Vsb[:, :, :, DH], 1.0)   # Z ones column

        bv3 = bv.rearrange("p (hp x d) -> p hp x d", x=2, d=DH)
        v3 = Vsb.rearrange("p t (hp x) e -> p t hp x e", x=2)

        # ---------------- LayerNorm helpers ----------------
        stats = sta.tile([128, TT, 8], fp32)   # _,_,mean,var,rstd,tmp,lnb,_

        def ln_stats_tile(xt, st):
            """per-tile mean/var via the fused BN_STATS path."""
            st6 = ldx.tile([128, 6], fp32, tag="st6", bufs=2)
            nc.vector.bn_stats(st6[:], xt)
            nc.vector.bn_aggr(st[:, 2:4], st6[:])

        def ln_group_rstd(sg):
            """batched (group) rstd via DVE Newton: sg [128, G, 8] with
            (mean, var) in cols 2,3. rstd -> col 4, lnb (=-mu*rstd) -> col 6."""
            mean, var = sg[:, :, 2], sg[:, :, 3]
            y, tmp, lnb_ = sg[:, :, 4], sg[:, :, 5], sg[:, :, 6]
            nc.vector.tensor_scalar_add(var, var, EPS)
            # y0 = 1 folded into first Newton step: y1 = 1.5 - 0.5*v
            nc.vector.tensor_scalar(
                y, var, -0.5, 1.5, op0=ALU.mult, op1=ALU.add)
            for _ in range(2):
                nc.vector.tensor_tensor(tmp, y, y, ALU.mult)
                nc.vector.tensor_tensor(tmp, tmp, var, ALU.mult)
                nc.vector.tensor_scalar(
                    tmp, tmp, -0.5, 1.5, op0=ALU.mult, op1=ALU.add)
                nc.vector.tensor_tensor(y, y, tmp, ALU.mult)
            nc.vector.tensor_tensor(tmp, mean, y, ALU.mult)
            nc.vector.tensor_scalar_mul(lnb_, tmp, -1.0)

        def ln_apply(xt, st, xn_out, on_act=False):
            if on_act:
                nc.scalar.activation(
                    xn_out, xt, AF.Identity, bias=st[:, 6:7], scale=st[:, 4:5])
            else:
                nc.vector.tensor_scalar(
                    xn_out, xt, st[:, 4:5], st[:, 6:7], op0=ALU.mult, op1=ALU.add)

        def transpose_to(xn, dstT, t, on_act=False):
            """3 PE transposes of xn [128, C] bf16 into dstT[:, :, t*128...]."""
            ptr = psg.tile([128, NF], bf16, tag="g", name="ptrA")
            for cs in range(CS):
                nc.tensor.transpose(
                    ptr[:, cs * 128:(cs + 1) * 128],
                    xn[:, cs * 128:(cs + 1) * 128], ident[:])
            src = ptr[:, :CS * 128].rearrange("p (cs n) -> p cs n", n=128)
            dst = dstT[:, :, t * 128:(t + 1) * 128]
            if on_act:
                nc.scalar.copy(dst, src)
            else:
                nc.vector.tensor_copy(dst, src)

        def build_qk(m, n):
            """one [128, NF] output tile of the fused Q/K projection.
            m in [0, HP): Q rows for head-pair m; m in [HP, 2 HP): K rows."""
            f = QF if m < HP else NF
            pk = psg.tile([128, NF], fp32, tag="g", name="pk")
            for cs in range(CS):
                nc.tensor.matmul(
                    pk[:, :f],
                    wqk[:, cs, m * 128:(m + 1) * 128],
                    xnT[:, cs, n * f:(n + 1) * f],
                    start=(cs == 0), stop=(cs == CS - 1))
            if m < HP:
                dst = QT[:, m, n * f:(n + 1) * f]
            else:
                dst = KT[:, m - HP, n * f:(n + 1) * f]
            nc.vector.tensor_scalar_add(dst, pk[:, :f], bias[:, m:m + 1])

        # ---------------- phase A: LN1 + x^T + V + K/Q builds ----------------
        G = min(4, TT)
        xtiles = {}

        def phase_a_ln(g):
            g0 = g * G
            for t in range(g0, g0 + G):
                xt = (x_own if t < QTT else x_oth)[:, t % QTT, :]
                xtiles[t] = xt
                nc.sync.dma_start(xt, xin_t[:, t, :])
                ln_stats_tile(xt, stats[:, t, :])
            ln_group_rstd(stats[:, g0:g0 + G, :])
            for t in range(g0, g0 + G):
                xn = xnp.tile([128, C], bf16, tag="xn")
                ln_apply(xtiles[t], stats[:, t, :], xn[:], on_act=True)
                transpose_to(xn, xnT, t, on_act=True)

        def phase_a_qk(g):
            for hp in range(HP):
                build_qk(HP + hp, g)        # K rows for this n-range
            if g == 0:
                for m in range(HP):
                    build_qk(m, 0)          # Q rows for j = 0

        def build_v(t):
            pv = psg.tile([128, NF], fp32, tag="g", name="pvA")
            for cs in range(CS):
                nc.tensor.matmul(
                    pv[:, :C], xnT[:, cs, t * 128:(t + 1) * 128],
                    wv[:, cs, :],
                    start=(cs == 0), stop=(cs == CS - 1))
            pv3 = pv[:, :C].rearrange("p (hp x d) -> p hp x d", x=2, d=DH)
            nc.vector.tensor_tensor(
                v3[:, t, :, :, :DH], pv3[:, :, :, :],
                bv3[:, :, :, :], ALU.add)

        # ---------------- attention (head-pair packed) ----------------
        def pv_pair(po, ech, h, p):
            # fp8 DoubleRow: contract a PAIR of kt tiles per matmul.
            nc.tensor.matmul(
                po[:DH + 1, :], Vsb[:, 2 * p:2 * p + 2, h, :DH + 1],
                ech[:, 2 * p:2 * p + 2, h % 2, :],
                start=(p == 0), stop=(p == NPV - 1),
                perf_mode=mybir.MatmulPerfMode.DoubleRow)

        def finalize_head(po, h, hp, j):
            # Z row (PSUM lane 64) -> SBUF lane 0 -> fast reciprocal ->
            # gpsimd partition-broadcast -> normalize-evacuate to AT.
            # custom-DVE ops and partition_broadcast only run at partition
            # base 0; the standard copy does the lane shift.
            hb = (h % 2) * 64
            z0 = rzp.tile([1, QF], fp32, tag="rz")
            nc.vector.tensor_copy(z0[:], po[64:65, :])
            rzr = rzp.tile([1, QF], fp32, tag="rzr")
            nc.vector.reciprocal_approx_fast(out=rzr[:], in_=z0[:])
            rzb = rzp.tile([64, QF], fp32, tag="rzb")
            nc.gpsimd.partition_broadcast(rzb[:], rzr[:])
            nc.vector.tensor_tensor(
                AT[hb:hb + 64, hp, j * QF:(j + 1) * QF],
                po[:64, :], rzb[:], ALU.mult)

        def attn_start():
            po_e = pom.tile([128, QF], fp32, tag="po", name="poE")
            po_o = pom.tile([128, QF], fp32, tag="po", name="poO")
            ech = echp.tile([128, TT, 2, QF], fp8, tag="ech")
            return po_e, po_o, ech

        def attn_chunks(st, hp, j, kts, do_pv=True):
            po_e, po_o, ech = st
            he, ho = 2 * hp, 2 * hp + 1
            jq = slice(j * QF, (j + 1) * QF)
            for kt in kts:
                psS = pss.tile([128, 2 * QF], fp32, tag="ss")
                # even head in PE rows 0-63, odd head in rows 64-127:
                # adjacent issue -> concurrent row-group execution
                nc.tensor.matmul(
                    psS[:, :QF], KT[0:64, hp, kt * 128:(kt + 1) * 128],
                    QT[0:64, hp, jq], start=True, stop=True)
                nc.tensor.matmul(
                    psS[:, QF:], KT[64:128, hp, kt * 128:(kt + 1) * 128],
                    QT[64:128, hp, jq], start=True, stop=True)
                nc.scalar.activation(ech[:, kt, :, :], psS[:], AF.Exp)
                # PV for kt-pair p trails by one pair so PE never
                # head-of-line blocks on the exp it needs
                if do_pv and kt % 2 == 1 and kt >= 3:
                    p = (kt - 3) // 2
                    pv_pair(po_e, ech, he, p)
                    pv_pair(po_o, ech, ho, p)

        def attn_finish(st, hp, j, p0=NPV - 1):
            po_e, po_o, ech = st
            he, ho = 2 * hp, 2 * hp + 1
            for p in range(p0, NPV):
                pv_pair(po_e, ech, he, p)
                pv_pair(po_o, ech, ho, p)
            finalize_head(po_e, he, hp, j)
            finalize_head(po_o, ho, hp, j)

        def attention_pair(hp, j):
            st = attn_start()
            attn_chunks(st, hp, j, range(TT))
            attn_finish(st, hp, j)

        def transpose_add(y_sb, dst, res):
            # y_sb [128, NB*128] bf16 -> PE transpose -> dst = res + y^T
            ptr = psg.tile([128, NF], bf16, tag="g", name="ptrC")
            for b in range(NB):
                nc.tensor.transpose(
                    ptr[:, b * 128:(b + 1) * 128],
                    y_sb[:, b * 128:(b + 1) * 128], ident[:])
            nc.vector.tensor_tensor(
                dst, ptr[:, :NB * 128].rearrange("p (b n) -> p b n", n=128),
                res, ALU.add)

        def proj_j(j, tail=False):
            t0 = j * NB
            for m in range(CS):
                if tail:
                    pp = pom.tile([128, QF], fp32, tag="po", name="pp")
                else:
                    pp = psg.tile([128, NF], fp32, tag="g", name="pp")
                for hp in range(HP):
                    nc.tensor.matmul(
                        pp[:, :QF], wp[:, hp, m * 128:(m + 1) * 128],
                        AT[:, hp, j * QF:(j + 1) * QF],
                        start=(hp == 0), stop=(hp == HP - 1))
                y1T = ytp.tile([128, QF], bf16, tag="yT")
                nc.vector.tensor_scalar_add(
                    y1T[:], pp[:, :QF], bias[:, 6 + m:7 + m])
                transpose_add(
                    y1T,
                    x2[:, t0:t0 + NB, m * 128:(m + 1) * 128],
                    x_own[:, t0:t0 + NB, m * 128:(m + 1) * 128])

        def ln2_j(j):
            t0 = j * NB
            for t in range(t0, t0 + NB):
                ln_stats_tile(x2[:, t, :], stats[:, t, :])
            ln_group_rstd(stats[:, t0:t0 + NB, :])
            for t in range(t0, t0 + NB):
                xn2 = xnp.tile([128, C], bf16, tag="xn")
                ln_apply(x2[:, t, :], stats[:, t, :], xn2[:])
                transpose_to(xn2, xn2T, t)

        def mlp_fc1_j(j, tail=False):
            hraw = hfp.tile([128, KS, QF], bf16, tag="hraw")
            for ks in range(KS):
                if tail:
                    pf1 = pss.tile([128, 2 * QF], fp32, tag="ss", name="pf1")
                else:
                    pf1 = psg.tile([128, NF], fp32, tag="g", name="pf1")
                for cs in range(CS):
                    nc.tensor.matmul(
                        pf1[:, :QF], wf1[:, cs, ks * 128:(ks + 1) * 128],
                        xn2T[:, cs, j * QF:(j + 1) * QF],
                        start=(cs == 0), stop=(cs == CS - 1))
                # stage raw fc1 in SBUF so the psum slot frees without an
                # ACT table switch; gelus for ALL j run as one batch after
                # the last exp (single exp->gelu table load, no thrash)
                nc.vector.tensor_copy(hraw[:, ks, :], pf1[:, :QF])
            return hraw

        def mlp_rest_j(j, hraw, tail=False):
            t0 = j * NB
            hful = hfp.tile([128, KS, QF], fp8, tag="hful")
            # one atomic gelu window: a single exp<->gelu table round-trip
            with tc.tile_critical():
                for ks in range(KS):
                    nc.scalar.activation(
                        hful[:, ks, :], hraw[:, ks, :], act_fn,
                        bias=bias[:, 9 + ks:10 + ks])
            for m in range(CS):
                if tail:
                    pf2 = pom.tile([128, QF], fp32, tag="po", name="pf2")
                else:
                    pf2 = psg.tile([128, NF], fp32, tag="g", name="pf2")
                for k2 in range(KS // 2):
                    nc.tensor.matmul(
                        pf2[:, :QF],
                        wf2[:, 2 * k2:2 * k2 + 2, m * 128:(m + 1) * 128],
                        hful[:, 2 * k2:2 * k2 + 2, :],
                        start=(k2 == 0), stop=(k2 == KS // 2 - 1),
                        perf_mode=mybir.MatmulPerfMode.DoubleRow)
                y2T = ytp.tile([128, QF], bf16, tag="yT")
                nc.vector.tensor_scalar(
                    y2T[:], pf2[:, :QF], 1.0 / WSCALE, bias[:, 21 + m:22 + m],
                    op0=ALU.mult, op1=ALU.add)
                transpose_add(
                    y2T,
                    x2[:, t0:t0 + NB, m * 128:(m + 1) * 128],
                    x2[:, t0:t0 + NB, m * 128:(m + 1) * 128])
            nc.sync.dma_start(
                yout_t[:, t0:t0 + NB, :], x2[:, t0:t0 + NB, :])

        # ---------------- main flow ----------------
        # phase-A groups interleaved with the earliest attention pairs so
        # the exp stream starts as soon as K/Q/V of group 0 exist; then
        # j-outer with proj/ln2/MLP of j-1 overlapping attention of j.
        NG = TT // G
        st00 = attn_start()
        for g in range(NG):
            phase_a_ln(g)
            if g == 0:
                # weight loads queue behind the group-0 x tiles so the
                # LN->K/Q critical path wins the DMA queues at startup
                nc.sync.dma_start(wqk[:], wqk_d.ap())
                nc.sync.dma_start(bias[:], bias_d.ap())
                nc.sync.dma_start(wv[:], wv_d.ap())
                nc.sync.dma_start(bv[:], bv_d.ap().to_broadcast([128, C]))
            phase_a_qk(g)
            if g == 1:
                # deferred weight loads (needed only from proj/MLP onward)
                nc.sync.dma_start(wp[:], wp_d.ap())
                nc.sync.dma_start(wf1[:], wf1_d.ap())
                nc.sync.dma_start(wf2[:], wf2_d.ap())
            # pair (0,0) S/exp chunks trail phase A group-by-group (Tile
            # semantics are program-order: reads must follow the writes);
            # its PVs defer past phase A so lean groups keep pace with exp
            attn_chunks(st00, 0, 0, range(g * G, g * G + G), do_pv=False)
        # V projections + leftover Q build + pair-0 PVs fill PE slack under
        # the exp stream of pairs 0-1
        for t in range(TT):
            build_v(t)
        for m in range(HP):
            build_qk(m, 1)
        attn_finish(st00, 0, 0, p0=0)
        for hp in range(1, HP):
            attention_pair(hp, 0)
        hraws = {}
        for j in range(1, NJ):
            attention_pair(0, j)
            proj_j(j - 1)
            ln2_j(j - 1)
            hraws[j - 1] = mlp_fc1_j(j - 1)
            for hp in range(1, HP):
                attention_pair(hp, j)
        proj_j(NJ - 1, tail=True)
        ln2_j(NJ - 1)
        hraws[NJ - 1] = mlp_fc1_j(NJ - 1, tail=True)
        for j in range(NJ):
            mlp_rest_j(j, hraws[j], tail=(j == NJ - 1))
    return nc


def prep_inputs(x, w_qkv, b_qkv, w_proj, b_proj, w_fc1, b_fc1, w_fc2, b_fc2,
                g1, beta1, g2, beta2, n_cores=8):
    """Host-side preprocessing: fold LN affine + attention scale into
    weights/biases, cast to bf16/fp8, reshape to SBUF layouts, permute x."""
    scale_q = DH ** -0.5

    wq = (g1[:, None] * w_qkv[:, :C]) * scale_q
    wk = g1[:, None] * w_qkv[:, C:2 * C]
    wv_ = g1[:, None] * w_qkv[:, 2 * C:]
    bq = (b_qkv[:C] + beta1 @ w_qkv[:, :C]) * scale_q
    bk = b_qkv[C:2 * C] + beta1 @ w_qkv[:, C:2 * C]
    bv_ = b_qkv[2 * C:] + beta1 @ w_qkv[:, 2 * C:]
    wf1_ = g2[:, None] * w_fc1
    bf1_ = b_fc1 + beta2 @ w_fc1

    def kx(w, dt=ml_dtypes.bfloat16):
        n = w.shape[0] // 128
        return np.ascontiguousarray(
            w.reshape(n, 128, w.shape[1]).transpose(1, 0, 2)
        ).astype(dt)

    wqk_l = kx(np.concatenate([wq, wk], axis=1))
    wv_l = kx(wv_)
    wp_l = kx(w_proj)
    wf1_l = kx(wf1_)
    wf2_l = kx(w_fc2 * WSCALE, ml_dtypes.float8_e4m3)

    bias_h = np.zeros((128, NBIAS), np.float32)
    bias_h[:, 0:3] = bq.reshape(3, 128).T
    bias_h[:, 3:6] = bk.reshape(3, 128).T
    bias_h[:, 6:9] = b_proj.reshape(3, 128).T
    bias_h[:, 9:21] = bf1_.reshape(12, 128).T
    bias_h[:, 21:24] = b_fc2.reshape(3, 128).T
    bv_l = np.ascontiguousarray(bv_.reshape(1, C), dtype=np.float32)

    B, N, _ = x.shape
    half = N // 2
    in_maps = []
    for core in range(n_cores):
        b, hf = core // 2, core % 2
        own = x[b, hf * half:(hf + 1) * half]
        other = x[b, (1 - hf) * half:(2 - hf) * half]
        xin_core = np.ascontiguousarray(
            np.concatenate([own, other], axis=0), dtype=np.float32)
        in_maps.append({
            "xin": xin_core, "wqk": wqk_l, "wv": wv_l, "wp": wp_l,
            "wf1": wf1_l, "wf2": wf2_l, "bias": bias_h, "bv": bv_l,
            f"bustin_{BUST}": np.zeros((1, 8), np.float32),
        })
    return in_maps


def assemble_output(results, B, N):
    half = N // 2
    y = np.empty((B, N, C), np.float32)
    for core, r in enumerate(results):
        b, hf = core // 2, core % 2
        y[b, hf * half:(hf + 1) * half] = r["yout"]
    return y


_CACHED = {}


def _get_compiled(SEQ):
    if SEQ not in _CACHED:
        from concourse import bacc
        nc = bacc.Bacc("TRN2", target_bir_lowering=False, debug=False)
        build(nc, SEQ=SEQ)
        nc.compile()
        _CACHED[SEQ] = nc
    return _CACHED[SEQ]


def kernel(x, w_qkv, b_qkv, w_proj, b_proj, w_fc1, b_fc1, w_fc2, b_fc2,
           g1, beta1, g2, beta2):
    from concourse.bass_utils import run_bass_kernel_spmd

    x = np.asarray(x, dtype=np.float32)
    B, N, _ = x.shape
    nc = _get_compiled(N)
    in_maps = prep_inputs(
        x, np.asarray(w_qkv, np.float32), np.asarray(b_qkv, np.float32),
        np.asarray(w_proj, np.float32), np.asarray(b_proj, np.float32),
        np.asarray(w_fc1, np.float32), np.asarray(b_fc1, np.float32),
        np.asarray(w_fc2, np.float32), np.asarray(b_fc2, np.float32),
        np.asarray(g1, np.float32), np.asarray(beta1, np.float32),
        np.asarray(g2, np.float32), np.asarray(beta2, np.float32),
        n_cores=2 * B)
    res = run_bass_kernel_spmd(
        nc, in_maps, core_ids=list(range(2 * B)), trace=False)
    return assemble_output(res.results, B=B, N=N)


# revision 31
# speedup vs baseline: 1.3457x; 1.3457x over previous
"""TRN2 Bass/Tile kernel for nn_Block_19756849561899 (pre-LN transformer
block: LN -> MHA -> residual -> LN -> MLP(gelu) -> residual).

Self-contained: kernel(**inputs) takes the full fp32 tensors, shards work
across 8 NeuronCores (one batch per core-pair; each core owns half the
sequence as queries and redundantly builds K/V for its batch), compiles a
Bass/Tile program once per process, runs it SPMD, and reassembles the full
output.

Structure (v2): the attention phase is ACT-bound (back-to-back EXPs pace
it), so MLP/proj PE work for query-tile j-1 is emitted to overlap the
attention of j. Scores for the two heads of a pair run concurrently in
disjoint PE row groups (contract dim is 64). All transposes ride the idle
DMA engines (xbar transpose) instead of PE+PSUM.
"""

import contextlib

import numpy as np
import ml_dtypes

import concourse.bass as bass
import concourse.mybir as mybir
import concourse.tile as tile
from concourse.masks import make_identity

fp32 = mybir.dt.float32
bf16 = mybir.dt.bfloat16
fp8 = mybir.dt.float8e4
AF = mybir.ActivationFunctionType
ALU = mybir.AluOpType

C = 384
CS = 3          # C / 128
H = 6
HP = 3          # head pairs
DH = 64
HID = 1536
KS = 12         # HID / 128
VW = 72         # padded V row width (DoubleRow needs 16B-aligned pair stride)
WSCALE = 16.0   # fp8 fc2 weight scaling (avoids e4m3 denormals)
EPS = 1e-6
NBIAS = 24
BUST = "b9"     # bump on every IR change: the NEFF cache key can miss
                # SBUF-internal IR edits, so keep the io signature fresh


def build(nc, SEQ=2048, act_fn=AF.Gelu):
    TT = SEQ // 128          # token tiles over full sequence
    QTT = TT // 2            # token tiles in own (query) half
    QLEN = SEQ // 2
    QF = min(512, QLEN)      # q free-dim tile
    NJ = QLEN // QF
    NF = min(512, SEQ)       # seq free-dim tile for K^T build
    NN = SEQ // NF
    NB = QF // 128           # token blocks per q-tile
    NPV = TT // 2            # PV DoubleRow kt-pairs

    xin = nc.dram_tensor("xin", [SEQ, C], fp32, kind="ExternalInput")
    wqk_d = nc.dram_tensor("wqk", [128, CS, 768], bf16, kind="ExternalInput")
    wv_d = nc.dram_tensor("wv", [128, CS, C], bf16, kind="ExternalInput")
    wp_d = nc.dram_tensor("wp", [128, CS, C], bf16, kind="ExternalInput")
    wf1_d = nc.dram_tensor("wf1", [128, CS, HID], bf16, kind="ExternalInput")
    wf2_d = nc.dram_tensor("wf2", [128, KS, C], fp8, kind="ExternalInput")
    bias_d = nc.dram_tensor("bias", [128, NBIAS], fp32, kind="ExternalInput")
    bv_d = nc.dram_tensor("bv", [1, C], fp32, kind="ExternalInput")
    yout = nc.dram_tensor("yout", [QLEN, C], fp32, kind="ExternalOutput")
    bust_d = nc.dram_tensor(f"bustin_{BUST}", [1, 8], fp32, kind="ExternalInput")
    bust_o = nc.dram_tensor(f"bustout_{BUST}", [1, 8], fp32, kind="ExternalOutput")

    xin_t = xin.ap().rearrange("(t p) c -> p t c", p=128)     # [128, TT, C]
    yout_t = yout.ap().rearrange("(t p) c -> p t c", p=128)   # [128, QTT, C]

    with tile.TileContext(nc) as tc, contextlib.ExitStack() as ctx:
        per = ctx.enter_context(tc.tile_pool(name="per", bufs=1))
        ldx = ctx.enter_context(tc.tile_pool(name="ldx", bufs=6))
        xnp = ctx.enter_context(tc.tile_pool(name="xnp", bufs=6))
        echp = ctx.enter_context(tc.tile_pool(name="echp", bufs=2))
        rzp = ctx.enter_context(tc.tile_pool(name="rzp", bufs=3))
        ytp = ctx.enter_context(tc.tile_pool(name="ytp", bufs=4))
        hfp = ctx.enter_context(tc.tile_pool(name="hfp", bufs=2))
        sta = ctx.enter_context(tc.tile_pool(name="sta", bufs=1))
        # PSUM banks: pss 2x2 (S head-pair chunks) + pom 2x1 (PV accum)
        #           + psg 2x1 (warm/QK/V/proj/MLP/transposes) = 8
        pss = ctx.enter_context(tc.tile_pool(name="pss", bufs=2, space="PSUM"))
        pom = ctx.enter_context(tc.tile_pool(name="pom", bufs=2, space="PSUM"))
        psg = ctx.enter_context(tc.tile_pool(name="psg", bufs=2, space="PSUM"))

        bust_t = per.tile([1, 8], fp32)
        nc.sync.dma_start(bust_t[:], bust_d.ap())
        nc.sync.dma_start(bust_o.ap(), bust_t[:])

        wqk = per.tile([128, CS, 768], bf16)
        wv = per.tile([128, CS, C], bf16)
        bias = per.tile([128, NBIAS], fp32)
        bv = per.tile([128, C], fp32)
        wp = per.tile([128, CS, C], bf16)
        wf1 = per.tile([128, CS, HID], bf16)
        wf2 = per.tile([128, KS, C], fp8)
        ident = per.tile([128, 128], bf16)
        make_identity(nc, ident)

        x_own = per.tile([128, QTT, C], fp32)
        x_oth = per.tile([128, QTT, C], fp32)
        x2 = per.tile([128, QTT, C], fp32)
        KT = per.tile([128, HP, SEQ], bf16)
        QT = per.tile([128, HP, QLEN], bf16)
        Vsb = per.tile([128, TT, H, VW], fp8)
        xnT = per.tile([128, CS, SEQ], bf16)
        xn2T = per.tile([128, CS, QLEN], bf16)
        AT = per.tile([128, HP, QLEN], bf16)

        nc.vector.memset(Vsb[:, :, :, DH], 1.0)   # Z ones column

        bv3 = bv.rearrange("p (hp x d) -> p hp x d", x=2, d=DH)
        v3 = Vsb.rearrange("p t (hp x) e -> p t hp x e", x=2)

        # ---------------- LayerNorm helpers ----------------
        stats = sta.tile([128, TT, 8], fp32)   # _,_,mean,var,rstd,tmp,lnb,_

        def ln_stats_tile(xt, st):
            """per-tile mean/var via the fused BN_STATS path."""
            st6 = ldx.tile([128, 6], fp32, tag="st6", bufs=2)
            nc.vector.bn_stats(st6[:], xt)
            nc.vector.bn_aggr(st[:, 2:4], st6[:])

        def ln_group_rstd(sg):
            """batched (group) rstd via DVE Newton: sg [128, G, 8] with
            (mean, var) in cols 2,3. rstd -> col 4, lnb (=-mu*rstd) -> col 6."""
            mean, var = sg[:, :, 2], sg[:, :, 3]
            y, tmp, lnb_ = sg[:, :, 4], sg[:, :, 5], sg[:, :, 6]
            nc.vector.tensor_scalar_add(var, var, EPS)
            # y0 = 1 folded into first Newton step: y1 = 1.5 - 0.5*v
            nc.vector.tensor_scalar(
                y, var, -0.5, 1.5, op0=ALU.mult, op1=ALU.add)
            for _ in range(2):
                nc.vector.tensor_tensor(tmp, y, y, ALU.mult)
                nc.vector.tensor_tensor(tmp, tmp, var, ALU.mult)
                nc.vector.tensor_scalar(
                    tmp, tmp, -0.5, 1.5, op0=ALU.mult, op1=ALU.add)
                nc.vector.tensor_tensor(y, y, tmp, ALU.mult)
            nc.vector.tensor_tensor(tmp, mean, y, ALU.mult)
            nc.vector.tensor_scalar_mul(lnb_, tmp, -1.0)

        def ln_apply(xt, st, xn_out, on_act=False):
            if on_act:
                nc.scalar.activation(
                    xn_out, xt, AF.Identity, bias=st[:, 6:7], scale=st[:, 4:5])
            else:
                nc.vector.tensor_scalar(
                    xn_out, xt, st[:, 4:5], st[:, 6:7], op0=ALU.mult, op1=ALU.add)

        def transpose_to(xn, dstT, t, on_act=False):
            """3 PE transposes of xn [128, C] bf16 into dstT[:, :, t*128...]."""
            ptr = psg.tile([128, NF], bf16, tag="g", name="ptrA")
            for cs in range(CS):
                nc.tensor.transpose(
                    ptr[:, cs * 128:(cs + 1) * 128],
                    xn[:, cs * 128:(cs + 1) * 128], ident[:])
            src = ptr[:, :CS * 128].rearrange("p (cs n) -> p cs n", n=128)
            dst = dstT[:, :, t * 128:(t + 1) * 128]
            if on_act:
                nc.scalar.copy(dst, src)
            else:
                nc.vector.tensor_copy(dst, src)

        def build_qk(m, n):
            """one [128, NF] output tile of the fused Q/K projection.
            m in [0, HP): Q rows for head-pair m; m in [HP, 2 HP): K rows."""
            f = QF if m < HP else NF
            pk = psg.tile([128, NF], fp32, tag="g", name="pk")
            for cs in range(CS):
                nc.tensor.matmul(
                    pk[:, :f],
                    wqk[:, cs, m * 128:(m + 1) * 128],
                    xnT[:, cs, n * f:(n + 1) * f],
                    start=(cs == 0), stop=(cs == CS - 1))
            if m < HP:
                dst = QT[:, m, n * f:(n + 1) * f]
            else:
                dst = KT[:, m - HP, n * f:(n + 1) * f]
            # ACT evac: phase-A DVE is loaded with bn/apply/evac work
            nc.scalar.add(dst, pk[:, :f], bias[:, m:m + 1])

        # ---------------- phase A: LN1 + x^T + V + K/Q builds ----------------
        G = min(4, TT)
        xtiles = {}

        def phase_a_ln(g):
            g0 = g * G
            for t in range(g0, g0 + G):
                xt = (x_own if t < QTT else x_oth)[:, t % QTT, :]
                xtiles[t] = xt
                nc.sync.dma_start(xt, xin_t[:, t, :])
                ln_stats_tile(xt, stats[:, t, :])
            ln_group_rstd(stats[:, g0:g0 + G, :])
            for t in range(g0, g0 + G):
                xn = xnp.tile([128, C], bf16, tag="xn")
                ln_apply(xtiles[t], stats[:, t, :], xn[:])
                transpose_to(xn, xnT, t)

        def phase_a_qk(g):
            for hp in range(HP):
                build_qk(HP + hp, g)        # K rows for this n-range
            if g == 0:
                for m in range(HP):
                    build_qk(m, 0)          # Q rows for j = 0

        def build_v(t):
            pv = psg.tile([128, NF], fp32, tag="g", name="pvA")
            for cs in range(CS):
                nc.tensor.matmul(
                    pv[:, :C], xnT[:, cs, t * 128:(t + 1) * 128],
                    wv[:, cs, :],
                    start=(cs == 0), stop=(cs == CS - 1))
            pv3 = pv[:, :C].rearrange("p (hp x d) -> p hp x d", x=2, d=DH)
            nc.vector.tensor_tensor(
                v3[:, t, :, :, :DH], pv3[:, :, :, :],
                bv3[:, :, :, :], ALU.add)

        # ---------------- attention (head-pair packed) ----------------
        def pv_pair(po, ech, h, p):
            # fp8 DoubleRow: contract a PAIR of kt tiles per matmul.
            nc.tensor.matmul(
                po[:DH + 1, :], Vsb[:, 2 * p:2 * p + 2, h, :DH + 1],
                ech[:, 2 * p:2 * p + 2, h % 2, :],
                start=(p == 0), stop=(p == NPV - 1),
                perf_mode=mybir.MatmulPerfMode.DoubleRow)

        def finalize_head(po, h, hp, j):
            # Z row (PSUM lane 64) -> SBUF lane 0 -> fast reciprocal ->
            # gpsimd partition-broadcast -> normalize-evacuate to AT.
            # custom-DVE ops and partition_broadcast only run at partition
            # base 0; the standard copy does the lane shift.
            hb = (h % 2) * 64
            z0 = rzp.tile([1, QF], fp32, tag="rz")
            nc.vector.tensor_copy(z0[:], po[64:65, :])
            rzr = rzp.tile([1, QF], fp32, tag="rzr")
            nc.vector.reciprocal_approx_fast(out=rzr[:], in_=z0[:])
            rzb = rzp.tile([64, QF], fp32, tag="rzb")
            nc.gpsimd.partition_broadcast(rzb[:], rzr[:])
            nc.vector.tensor_tensor(
                AT[hb:hb + 64, hp, j * QF:(j + 1) * QF],
                po[:64, :], rzb[:], ALU.mult)

        def attn_start():
            po_e = pom.tile([128, QF], fp32, tag="po", name="poE")
            po_o = pom.tile([128, QF], fp32, tag="po", name="poO")
            ech = echp.tile([128, TT, 2, QF], fp8, tag="ech")
            return po_e, po_o, ech

        def attn_chunks(st, hp, j, kts, do_pv=True):
            po_e, po_o, ech = st
            he, ho = 2 * hp, 2 * hp + 1
            jq = slice(j * QF, (j + 1) * QF)
            for kt in kts:
                psS = pss.tile([128, 2 * QF], fp32, tag="ss")
                # even head in PE rows 0-63, odd head in rows 64-127:
                # adjacent issue -> concurrent row-group execution
                nc.tensor.matmul(
                    psS[:, :QF], KT[0:64, hp, kt * 128:(kt + 1) * 128],
                    QT[0:64, hp, jq], start=True, stop=True)
                nc.tensor.matmul(
                    psS[:, QF:], KT[64:128, hp, kt * 128:(kt + 1) * 128],
                    QT[64:128, hp, jq], start=True, stop=True)
                nc.scalar.activation(ech[:, kt, :, :], psS[:], AF.Exp)
                # PV for kt-pair p trails by one pair so PE never
                # head-of-line blocks on the exp it needs
                if do_pv and kt % 2 == 1 and kt >= 3:
                    p = (kt - 3) // 2
                    pv_pair(po_e, ech, he, p)
                    pv_pair(po_o, ech, ho, p)

        def attn_finish(st, hp, j, p0=NPV - 1):
            po_e, po_o, ech = st
            he, ho = 2 * hp, 2 * hp + 1
            for p in range(p0, NPV):
                pv_pair(po_e, ech, he, p)
                pv_pair(po_o, ech, ho, p)
            finalize_head(po_e, he, hp, j)
            finalize_head(po_o, ho, hp, j)

        def attention_pair(hp, j):
            st = attn_start()
            attn_chunks(st, hp, j, range(TT))
            attn_finish(st, hp, j)

        def transpose_add(y_sb, dst, res):
            # y_sb [128, NB*128] bf16 -> PE transpose -> dst = res + y^T
            ptr = psg.tile([128, NF], bf16, tag="g", name="ptrC")
            for b in range(NB):
                nc.tensor.transpose(
                    ptr[:, b * 128:(b + 1) * 128],
                    y_sb[:, b * 128:(b + 1) * 128], ident[:])
            nc.vector.tensor_tensor(
                dst, ptr[:, :NB * 128].rearrange("p (b n) -> p b n", n=128),
                res, ALU.add)

        def proj_j(j, tail=False):
            t0 = j * NB
            for m in range(CS):
                if tail:
                    pp = pom.tile([128, QF], fp32, tag="po", name="pp")
                else:
                    pp = psg.tile([128, NF], fp32, tag="g", name="pp")
                for hp in range(HP):
                    nc.tensor.matmul(
                        pp[:, :QF], wp[:, hp, m * 128:(m + 1) * 128],
                        AT[:, hp, j * QF:(j + 1) * QF],
                        start=(hp == 0), stop=(hp == HP - 1))
                y1T = ytp.tile([128, QF], bf16, tag="yT")
                nc.vector.tensor_scalar_add(
                    y1T[:], pp[:, :QF], bias[:, 6 + m:7 + m])
                transpose_add(
                    y1T,
                    x2[:, t0:t0 + NB, m * 128:(m + 1) * 128],
                    x_own[:, t0:t0 + NB, m * 128:(m + 1) * 128])

        def ln2_j(j):
            t0 = j * NB
            for t in range(t0, t0 + NB):
                ln_stats_tile(x2[:, t, :], stats[:, t, :])
            ln_group_rstd(stats[:, t0:t0 + NB, :])
            for t in range(t0, t0 + NB):
                xn2 = xnp.tile([128, C], bf16, tag="xn")
                ln_apply(x2[:, t, :], stats[:, t, :], xn2[:])
                transpose_to(xn2, xn2T, t)

        def mlp_fc1_j(j, tail=False):
            hraw = hfp.tile([128, KS, QF], bf16, tag="hraw")
            for ks in range(KS):
                if tail:
                    pf1 = pss.tile([128, 2 * QF], fp32, tag="ss", name="pf1")
                else:
                    pf1 = psg.tile([128, NF], fp32, tag="g", name="pf1")
                for cs in range(CS):
                    nc.tensor.matmul(
                        pf1[:, :QF], wf1[:, cs, ks * 128:(ks + 1) * 128],
                        xn2T[:, cs, j * QF:(j + 1) * QF],
                        start=(cs == 0), stop=(cs == CS - 1))
                # stage raw fc1 in SBUF so the psum slot frees without an
                # ACT table switch; gelus for ALL j run as one batch after
                # the last exp (single exp->gelu table load, no thrash)
                nc.vector.tensor_copy(hraw[:, ks, :], pf1[:, :QF])
            return hraw

        def mlp_rest_j(j, hraw, tail=False):
            t0 = j * NB
            hful = hfp.tile([128, KS, QF], fp8, tag="hful")
            for ks in range(KS):
                nc.scalar.activation(
                    hful[:, ks, :], hraw[:, ks, :], act_fn,
                    bias=bias[:, 9 + ks:10 + ks])
            for m in range(CS):
                if tail:
                    pf2 = pom.tile([128, QF], fp32, tag="po", name="pf2")
                else:
                    pf2 = psg.tile([128, NF], fp32, tag="g", name="pf2")
                for k2 in range(KS // 2):
                    nc.tensor.matmul(
                        pf2[:, :QF],
                        wf2[:, 2 * k2:2 * k2 + 2, m * 128:(m + 1) * 128],
                        hful[:, 2 * k2:2 * k2 + 2, :],
                        start=(k2 == 0), stop=(k2 == KS // 2 - 1),
                        perf_mode=mybir.MatmulPerfMode.DoubleRow)
                y2T = ytp.tile([128, QF], bf16, tag="yT")
                nc.vector.tensor_scalar(
                    y2T[:], pf2[:, :QF], 1.0 / WSCALE, bias[:, 21 + m:22 + m],
                    op0=ALU.mult, op1=ALU.add)
                transpose_add(
                    y2T,
                    x2[:, t0:t0 + NB, m * 128:(m + 1) * 128],
                    x2[:, t0:t0 + NB, m * 128:(m + 1) * 128])
            nc.sync.dma_start(
                yout_t[:, t0:t0 + NB, :], x2[:, t0:t0 + NB, :])

        # ---------------- main flow ----------------
        # phase-A groups interleaved with the earliest attention pairs so
        # the exp stream starts as soon as K/Q/V of group 0 exist; then
        # j-outer with proj/ln2/MLP of j-1 overlapping attention of j.
        NG = TT // G
        st00 = attn_start()
        for g in range(NG):
            phase_a_ln(g)
            if g == 0:
                # weight loads queue behind the group-0 x tiles so the
                # LN->K/Q critical path wins the DMA queues at startup
                nc.sync.dma_start(wqk[:], wqk_d.ap())
                nc.sync.dma_start(bias[:], bias_d.ap())
                nc.sync.dma_start(wv[:], wv_d.ap())
                nc.sync.dma_start(bv[:], bv_d.ap().to_broadcast([128, C]))
            phase_a_qk(g)
            if g == 1:
                # deferred weight loads (needed only from proj/MLP onward)
                nc.sync.dma_start(wp[:], wp_d.ap())
                nc.sync.dma_start(wf1[:], wf1_d.ap())
                nc.sync.dma_start(wf2[:], wf2_d.ap())
            # pair (0,0) S/exp chunks trail phase A group-by-group (Tile
            # semantics are program-order: reads must follow the writes);
            # its PVs defer past phase A so lean groups keep pace with exp
            attn_chunks(st00, 0, 0, range(g * G, g * G + G), do_pv=False)
        # V projections + leftover Q build + pair-0 PVs fill PE slack under
        # the exp stream of pairs 0-1
        for t in range(TT):
            build_v(t)
        for m in range(HP):
            build_qk(m, 1)
        attn_finish(st00, 0, 0, p0=0)
        for hp in range(1, HP):
            attention_pair(hp, 0)
        hraws = {}
        for j in range(1, NJ):
            attention_pair(0, j)
            proj_j(j - 1)
            ln2_j(j - 1)
            hraws[j - 1] = mlp_fc1_j(j - 1)
            for hp in range(1, HP):
                attention_pair(hp, j)
        proj_j(NJ - 1, tail=True)
        ln2_j(NJ - 1)
        hraws[NJ - 1] = mlp_fc1_j(NJ - 1, tail=True)
        for j in range(NJ):
            mlp_rest_j(j, hraws[j], tail=(j == NJ - 1))
    return nc


def prep_inputs(x, w_qkv, b_qkv, w_proj, b_proj, w_fc1, b_fc1, w_fc2, b_fc2,
                g1, beta1, g2, beta2, n_cores=8):
    """Host-side preprocessing: fold LN affine + attention scale into
    weights/biases, cast to bf16/fp8, reshape to SBUF layouts, permute x."""
    scale_q = DH ** -0.5

    wq = (g1[:, None] * w_qkv[:, :C]) * scale_q
    wk = g1[:, None] * w_qkv[:, C:2 * C]
    wv_ = g1[:, None] * w_qkv[:, 2 * C:]
    bq = (b_qkv[:C] + beta1 @ w_qkv[:, :C]) * scale_q
    bk = b_qkv[C:2 * C] + beta1 @ w_qkv[:, C:2 * C]
    bv_ = b_qkv[2 * C:] + beta1 @ w_qkv[:, 2 * C:]
    wf1_ = g2[:, None] * w_fc1
    bf1_ = b_fc1 + beta2 @ w_fc1

    def kx(w, dt=ml_dtypes.bfloat16):
        n = w.shape[0] // 128
        return np.ascontiguousarray(
            w.reshape(n, 128, w.shape[1]).transpose(1, 0, 2)
        ).astype(dt)

    wqk_l = kx(np.concatenate([wq, wk], axis=1))
    wv_l = kx(wv_)
    wp_l = kx(w_proj)
    wf1_l = kx(wf1_)
    wf2_l = kx(w_fc2 * WSCALE, ml_dtypes.float8_e4m3)

    bias_h = np.zeros((128, NBIAS), np.float32)
    bias_h[:, 0:3] = bq.reshape(3, 128).T
    bias_h[:, 3:6] = bk.reshape(3, 128).T
    bias_h[:, 6:9] = b_proj.reshape(3, 128).T
    bias_h[:, 9:21] = bf1_.reshape(12, 128).T
    bias_h[:, 21:24] = b_fc2.reshape(3, 128).T
    bv_l = np.ascontiguousarray(bv_.reshape(1, C), dtype=np.float32)

    B, N, _ = x.shape
    half = N // 2
    in_maps = []
    for core in range(n_cores):
        b, hf = core // 2, core % 2
        own = x[b, hf * half:(hf + 1) * half]
        other = x[b, (1 - hf) * half:(2 - hf) * half]
        xin_core = np.ascontiguousarray(
            np.concatenate([own, other], axis=0), dtype=np.float32)
        in_maps.append({
            "xin": xin_core, "wqk": wqk_l, "wv": wv_l, "wp": wp_l,
            "wf1": wf1_l, "wf2": wf2_l, "bias": bias_h, "bv": bv_l,
            f"bustin_{BUST}": np.zeros((1, 8), np.float32),
        })
    return in_maps


def assemble_output(results, B, N):
    half = N // 2
    y = np.empty((B, N, C), np.float32)
    for core, r in enumerate(results):
        b, hf = core // 2, core % 2
        y[b, hf * half:(hf + 1) * half] = r["yout"]
    return y


_CACHED = {}


def _get_compiled(SEQ):
    if SEQ not in _CACHED:
        from concourse import bacc
        nc = bacc.Bacc("TRN2", target_bir_lowering=False, debug=False)
        build(nc, SEQ=SEQ)
        nc.compile()
        _CACHED[SEQ] = nc
    return _CACHED[SEQ]


def kernel(x, w_qkv, b_qkv, w_proj, b_proj, w_fc1, b_fc1, w_fc2, b_fc2,
           g1, beta1, g2, beta2):
    from concourse.bass_utils import run_bass_kernel_spmd

    x = np.asarray(x, dtype=np.float32)
    B, N, _ = x.shape
    nc = _get_compiled(N)
    in_maps = prep_inputs(
        x, np.asarray(w_qkv, np.float32), np.asarray(b_qkv, np.float32),
        np.asarray(w_proj, np.float32), np.asarray(b_proj, np.float32),
        np.asarray(w_fc1, np.float32), np.asarray(b_fc1, np.float32),
        np.asarray(w_fc2, np.float32), np.asarray(b_fc2, np.float32),
        np.asarray(g1, np.float32), np.asarray(beta1, np.float32),
        np.asarray(g2, np.float32), np.asarray(beta2, np.float32),
        n_cores=2 * B)
    res = run_bass_kernel_spmd(
        nc, in_maps, core_ids=list(range(2 * B)), trace=False)
    return assemble_output(res.results, B=B, N=N)
